# revision 47
# baseline (speedup 1.0000x reference)
"""HNetv1 Trainium2 Bass kernel (v2).

Strategy (8 NeuronCores):
  - All weights/inputs stream from HBM in fp8e3m4 (w1 x256, w2 x128, w3 x64,
    x1/x2 x2) -- halves the dominant w1 DMA vs bf16; rel-err ~4e-3 (gate 2e-2).
  - Every core computes the correlation for all 64 batches; normalization is
    asymmetric: r2 (x2-side norms) is broadcast via an all-ones matmul and
    pre-multiplied into x2; r1 (x1-side norms) is computed as per-ij psum
    columns and fused into the psum->X copy as an activation scale vector.
  - L1 (x[64,20736] @ w1-slice[20736,648]) is column-split 8 ways; per core the
    162 k-tile matmuls are 2x column-tiled on the PE array (even tiles ->
    psum partitions 0:64, odd -> 64:128) so the 64-batch stationary operand
    fills the whole array.  w1 is prefetched from t=0 in 6 linear chunks.
  - L2 is row-split (local 648 rows of w2, full 1296 cols, transposed output)
    followed by ONE AllReduce of the bf16 partials; L3/L4 run redundantly.
"""

import os
import numpy as np
import ml_dtypes

N, C, S = 64, 128, 12
HW = S * S              # 144
RIN = S ** 4            # 20736
NCORES = 8
COLS1 = 5184 // NCORES  # 648
NT1 = 162               # L1 k-tiles of 128
S1, S2, S3, SX = 256.0, 128.0, 64.0, 2.0
G = 4                   # batches per corr group
NGRP = N // G           # 16

_CACHE = {}
LAST_RESULT = None

_e3 = ml_dtypes.float8_e3m4
_bf = ml_dtypes.bfloat16


def _build_nc():
    import concourse.bacc as bacc
    import concourse.tile as tile
    import concourse.mybir as mybir
    from concourse.masks import make_identity

    dt = mybir.dt
    AF = mybir.ActivationFunctionType
    ALU = mybir.AluOpType

    nc = bacc.Bacc("TRN2", target_bir_lowering=False, debug=False,
                   num_devices=NCORES)

    x1q_d = nc.dram_tensor("x1q", [C, N, HW], dt.float8e3, kind="ExternalInput")
    x2q_d = nc.dram_tensor("x2q", [C, N, HW], dt.float8e3, kind="ExternalInput")
    w1t_d = nc.dram_tensor("w1t", [128, NT1, COLS1], dt.float8e3, kind="ExternalInput")
    w2t_d = nc.dram_tensor("w2t", [128, 6, 1296], dt.float8e3, kind="ExternalInput")
    w3t_d = nc.dram_tensor("w3t", [128, 11, 324], dt.float8e3, kind="ExternalInput")
    w4t_d = nc.dram_tensor("w4t", [128, 3, 8], dt.bfloat16, kind="ExternalInput")
    b1r_d = nc.dram_tensor("b1r", [1, COLS1], dt.bfloat16, kind="ExternalInput")
    b2r_d = nc.dram_tensor("b2r", [1, 1296], dt.bfloat16, kind="ExternalInput")
    b3r_d = nc.dram_tensor("b3r", [1, 324], dt.bfloat16, kind="ExternalInput")
    b4r_d = nc.dram_tensor("b4r", [1, 8], dt.bfloat16, kind="ExternalInput")
    out_d = nc.dram_tensor("out", [N, 8], dt.float32, kind="ExternalOutput")

    rg = [list(range(NCORES))]

    with tile.TileContext(nc) as tc:
        with tc.tile_pool(name="persist", bufs=1) as persist, \
             tc.tile_pool(name="dramp", bufs=1, space="DRAM") as dramp:
            ar_in = dramp.tile([128, 704], dt.bfloat16)
            ar_out = dramp.tile([128, 704], dt.bfloat16, addr_space="Shared")
            warm_in = dramp.tile([1, 16], dt.bfloat16)
            warm_out = dramp.tile([8, 16], dt.bfloat16, addr_space="Shared")

            ident = persist.tile([128, 128], dt.bfloat16)
            make_identity(nc, ident[:])
            ones128 = persist.tile([128, 128], dt.bfloat16)
            nc.vector.memset(ones128[:], 1.0)

            x1sb = persist.tile([C, N, HW], dt.float8e3)
            x2sb = persist.tile([C, N, HW], dt.float8e3)
            x2s = persist.tile([C, N, HW], dt.bfloat16)
            X_a = persist.tile([128, N, HW], dt.bfloat16)   # [ij, n, k]
            X_b = persist.tile([128, 18, N], dt.bfloat16)   # [16*bi+r, bo, n]
            X_btmp = persist.tile([16, HW, N], dt.bfloat16)
            r1col = persist.tile([128, N], dt.float32)
            r1colB = persist.tile([16, N], dt.float32)
            w2sb = persist.tile([128, 6, 1296], dt.float8e3)
            w3sb = persist.tile([128, 11, 324], dt.float8e3)
            w4sb = persist.tile([128, 3, 8], dt.bfloat16)
            b1row = persist.tile([1, COLS1], dt.bfloat16)
            b2row = persist.tile([1, 1296], dt.bfloat16)
            b3row = persist.tile([1, 324], dt.bfloat16)
            b4row = persist.tile([1, 8], dt.bfloat16)
            h1sb = persist.tile([64, COLS1], dt.bfloat16)
            h1T = persist.tile([128, 6, N], dt.bfloat16)
            arsb = persist.tile([128, 704], dt.bfloat16)
            ar2sb = persist.tile([128, 704], dt.bfloat16)
            h2T = persist.tile([128, 11, N], dt.bfloat16)
            h3sb = persist.tile([64, 324], dt.bfloat16)
            h3T = persist.tile([128, 3, N], dt.bfloat16)
            outsb = persist.tile([64, 8], dt.float32)

            nc.vector.memset(h1T[:], 0.0)
            nc.vector.memset(h2T[:], 0.0)
            nc.vector.memset(h3T[:], 0.0)

            # ---- input DMAs.  Sync(SP) HWDGE queue: x chunks first, then the
            # six w1 ring chunks (the ring pool lives across corr+L1 so the
            # first three stream during corr).  Scalar(ACT) HWDGE queue takes
            # the small tensors + later SBUF-SBUF regroup / collective DMAs so
            # a sem-gated w1 chunk at the SP queue head never blocks them.
            nc.sync.dma_start(x2sb[:, 0:32, :], x2q_d[:, 0:32, :])
            nc.sync.dma_start(x2sb[:, 32:64, :], x2q_d[:, 32:64, :])
            nc.sync.dma_start(x1sb[:], x1q_d[:, :, :])

            # warm up the collective path (startup barrier + first-use cost)
            # while corr/L1 run; gpsimd has nothing else queued before it.
            nc.gpsimd.collective_compute(
                "AllGather", mybir.AluOpType.bypass, replica_groups=rg,
                ins=[warm_in[:]], outs=[warm_out[:]])

            # w1 ring: issue all chunk DMAs now (SP queue, behind x) so
            # w1 streams during the corr phase; mms consume the tiles in L1.
            NCH, CHT = 9, 18
            w1pool = tc.tile_pool(name="w1p", bufs=8)
            w1p = w1pool.__enter__()
            w1tiles = []
            for ch in range(NCH):
                w1t = w1p.tile([128, CHT, COLS1], dt.float8e3, tag="w1t")
                nc.sync.dma_start(w1t[:], w1t_d[:, CHT * ch:CHT * ch + CHT, :])
                w1tiles.append(w1t)
            nc.sync.dma_start(w2sb[:], w2t_d[:, :, :])
            nc.sync.dma_start(w3sb[:], w3t_d[:, :, :])
            nc.sync.dma_start(w4sb[:], w4t_d[:, :, :])
            nc.sync.dma_start(b1row[:], b1r_d[:, :])
            nc.sync.dma_start(b2row[:], b2r_d[:, :])
            nc.sync.dma_start(b3row[:], b3r_d[:, :])
            nc.sync.dma_start(b4row[:], b4r_d[:, :])

            # ---------------- corr phase ----------------
            # Big-op streams: C1 normalizes x2 in groups of 8 (sq2 DVE,
            # 3 ssq-bcast mms, one rsqrt activation, one premult); C2 makes
            # x1 norm columns; C3 computes raw corr mms in rounds of 3
            # batches with single multi-batch scaled psum->X writes.
            G8 = 8
            with tc.tile_pool(name="csq", bufs=2) as csq, \
                 tc.tile_pool(name="crs", bufs=2) as crs, \
                 tc.tile_pool(name="pq2", bufs=1, space="PSUM") as pq2, \
                 tc.tile_pool(name="pca", bufs=2, space="PSUM") as pca, \
                 tc.tile_pool(name="pcb", bufs=2, space="PSUM") as pcb, \
                 tc.tile_pool(name="pr1", bufs=1, space="PSUM") as pr1:
                r1ps = pr1.tile([128, 128], dt.float32, tag="r1ps")
                for g in range(N // G8):
                    n0 = G8 * g
                    x2g = x2sb[:, n0:n0 + G8, :]
                    sq2 = csq.tile([C, G8, HW], dt.bfloat16, tag="sq2")
                    nc.vector.tensor_tensor(sq2[:], x2g, x2g, ALU.mult)
                    sq2f = sq2[:].rearrange("p g k -> p (g k)")
                    q2b = pq2.tile([128, 3, 512], dt.float32, tag="q2b")
                    for h in range(3):
                        nc.tensor.matmul(q2b[:, h, 0:384], ones128[:],
                                         sq2f[:, 384 * h:384 * h + 384],
                                         start=True, stop=True)
                    r2s = crs.tile([C, G8, HW], dt.bfloat16, tag="r2s")
                    nc.scalar.activation(
                        r2s[:].rearrange("p g k -> p (g k)").rearrange(
                            "p (h x) -> p h x", h=3),
                        q2b[:, :, 0:384], AF.Abs_reciprocal_sqrt)
                    nc.vector.tensor_tensor(x2s[:, n0:n0 + G8, :], x2g, r2s[:],
                                            ALU.mult)

                for g in range(N // G8):
                    n0 = G8 * g
                    sq1 = csq.tile([C, G8, HW], dt.bfloat16, tag="sq1")
                    nc.gpsimd.tensor_tensor(sq1[:], x1sb[:, n0:n0 + G8, :],
                                            x1sb[:, n0:n0 + G8, :], ALU.mult)
                    for j in range(G8):
                        n = n0 + j
                        nc.tensor.matmul(r1ps[:, n:n + 1], sq1[:, j, 0:128],
                                         ones128[:, 0:1], start=True, stop=True)
                        nc.tensor.matmul(r1ps[0:16, 64 + n:65 + n],
                                         sq1[:, j, 128:HW], ones128[:, 0:1],
                                         start=True, stop=True)
                    nc.scalar.activation(r1col[:, n0:n0 + G8],
                                         r1ps[:, n0:n0 + G8],
                                         AF.Abs_reciprocal_sqrt)
                    nc.scalar.activation(r1colB[:, n0:n0 + G8],
                                         r1ps[0:16, 64 + n0:64 + n0 + G8],
                                         AF.Abs_reciprocal_sqrt)

                nb = 0
                for r in range(22):
                    nn = nb
                    m = min(3, N - nn)
                    nb += m
                    ca = pca.tile([128, 512], dt.float32, tag="ca")
                    cb = pcb.tile([16, 512], dt.float32, tag="cb")
                    for j in range(m):
                        n = nn + j
                        nc.tensor.matmul(ca[:, HW * j:HW * j + HW],
                                         x1sb[:, n, 0:128], x2s[:, n, :],
                                         start=True, stop=True)
                        nc.tensor.matmul(cb[:, HW * j:HW * j + HW],
                                         x1sb[:, n, 128:HW], x2s[:, n, :],
                                         start=True, stop=True)
                    nc.vector.tensor_tensor(
                        X_a[:, nn:nn + m, :],
                        ca[:, 0:HW * m].rearrange("p (b k) -> p b k", k=HW),
                        r1col[:, nn:nn + m].broadcast_to([128, m, HW]),
                        ALU.mult)
                    for j in range(m):
                        nc.scalar.activation(
                            X_btmp[:, :, nn + j], cb[:, HW * j:HW * j + HW],
                            AF.Copy, scale=r1colB[:, nn + j:nn + j + 1])

                # regroup residue: X_b[16*bi+r, bo, n] = X_btmp[r, 8*bo+bi, n]
                xbt = X_btmp[:].rearrange("r (bo bi) n -> r bo bi n", bi=8)
                for bi in range(8):
                    nc.sync.dma_start(X_b[16 * bi:16 * bi + 16, :, :],
                                      xbt[:, :, bi, :])

            # ---------------- L1: col-tiled 2x, w1 streamed in a ring ----------------
            with tc.tile_pool(name="pl1", bufs=1, space="PSUM") as pl1, \
                 tc.tile_pool(name="l1t", bufs=1) as l1t:
                ps = [pl1.tile([128, 512], dt.float32, tag=f"ps{h}", name=f"ps{h}")
                      for h in range(2)]
                # issue order alternates PE column groups (A: psum rows 0:64,
                # B: 64:128) so paired tiles stream concurrently in the array
                for ch in range(NCH):
                    w1t = w1tiles[ch]
                    for uu in range(CHT // 2):
                        for h in range(2):
                            for v in range(2):
                                tt = 2 * uu + v
                                t = CHT * ch + tt
                                xt = (X_a[:, :, t] if t < 144
                                      else X_b[:, t - 144, :])
                                base = 64 * (t & 1)
                                nc.tensor.matmul(
                                    ps[h][base:base + 64, 0:324], xt,
                                    w1t[:, tt, 324 * h:324 * h + 324],
                                    start=(t < 2), stop=(t == NT1 - 1))
                for h in range(2):
                    nc.tensor.matmul(ps[h][0:64, 0:324], ones128[0:1, 0:64],
                                     b1row[:, 324 * h:324 * h + 324],
                                     start=False, stop=True)
                h1tmpB = l1t.tile([64, COLS1], dt.bfloat16, tag="h1tmpB")
                h1pre = l1t.tile([64, COLS1], dt.bfloat16, tag="h1pre")
                for h in range(2):
                    nc.scalar.activation(h1tmpB[:, 324 * h:324 * h + 324],
                                         ps[h][64:128, 0:324], AF.Copy,
                                         scale=1.0 / S1)
                    nc.vector.scalar_tensor_tensor(
                        h1pre[:, 324 * h:324 * h + 324], ps[h][0:64, 0:324],
                        1.0 / S1, h1tmpB[:, 324 * h:324 * h + 324],
                        ALU.mult, ALU.add)
                nc.vector.tensor_scalar_max(h1sb[:], h1pre[:], 0.0)

            # transpose h1 -> h1T [648(pad 768), 64]
            with tc.tile_pool(name="ptp", bufs=2, space="PSUM") as ptp:
                for t in range(6):
                    w = 128 if t < 5 else COLS1 - 5 * 128  # 8
                    tp = ptp.tile([128, 64], dt.bfloat16, tag="tp")
                    nc.tensor.transpose(tp[0:w, :], h1sb[:, 128 * t:128 * t + w],
                                        ident[0:64, 0:64])
                    nc.vector.tensor_copy(h1T[0:w, t, :], tp[0:w, :])

            # ---------------- L2 (row-split) + pre-AR transpose ----------------
            with tc.tile_pool(name="pl2", bufs=1, space="PSUM") as pl2, \
                 tc.tile_pool(name="l2t", bufs=1) as l2t, \
                 tc.tile_pool(name="ptp2", bufs=2, space="PSUM") as ptp2:
                h2p = [pl2.tile([64, 432], dt.float32, tag=f"h2p{q}",
                                name=f"h2p{q}") for q in range(3)]
                for kt in range(6):
                    for q in range(3):
                        nc.tensor.matmul(h2p[q][:, :],
                                         h1T[:, kt, :],
                                         w2sb[:, kt, 432 * q:432 * q + 432],
                                         start=(kt == 0), stop=False)
                for q in range(3):
                    nc.tensor.matmul(h2p[q][:, :], ones128[0:1, 0:64],
                                     b2row[0:1, 432 * q:432 * q + 432],
                                     start=False, stop=True)
                h2sb = l2t.tile([64, 1296], dt.bfloat16, tag="h2sb")
                for q in range(3):
                    nc.scalar.activation(h2sb[:, 432 * q:432 * q + 432],
                                         h2p[q][:, :], AF.Copy, scale=1.0 / S2)
                nc.vector.memset(arsb[:, 640:704], 0.0)
                for mt in range(11):
                    w = 128 if mt < 10 else 16
                    tp = ptp2.tile([128, 64], dt.bfloat16, tag="tp2")
                    nc.tensor.transpose(tp[0:w, :],
                                        h2sb[:, 128 * mt:128 * mt + w],
                                        ident[0:64, 0:64])
                    nc.vector.tensor_copy(
                        arsb[:, 64 * mt:64 * mt + 64][0:w, :], tp[0:w, :])
            nc.sync.dma_start(ar_in[:], arsb[:])
            nc.gpsimd.collective_compute(
                "AllReduce", mybir.AluOpType.add, replica_groups=rg,
                ins=[ar_in[:]], outs=[ar_out[:]])
            nc.sync.dma_start(ar2sb[:], ar_out[:])
            h2Tf = h2T[:].rearrange("p t n -> p (t n)")
            nc.vector.tensor_scalar_max(h2Tf[:, 0:704], ar2sb[:, 0:704], 0.0)

            # ---------------- L3 (redundant, col-tiled 2x) ----------------
            with tc.tile_pool(name="pl3", bufs=1, space="PSUM") as pl3, \
                 tc.tile_pool(name="l3t", bufs=1) as l3t:
                h3ps = pl3.tile([128, 512], dt.float32, tag="h3ps")
                for kt in range(11):
                    base = 64 * (kt & 1)
                    nc.tensor.matmul(h3ps[base:base + 64, 0:324], h2T[:, kt, :],
                                     w3sb[:, kt, :],
                                     start=(kt < 2), stop=(kt == 9))
                nc.tensor.matmul(h3ps[0:64, 0:324], ones128[0:1, 0:64], b3row[:],
                                 start=False, stop=True)
                h3tmpB = l3t.tile([64, 324], dt.bfloat16, tag="h3tmpB")
                nc.scalar.activation(h3tmpB[:], h3ps[64:128, 0:324], AF.Copy)
                h3pre = l3t.tile([64, 324], dt.float32, tag="h3pre")
                nc.vector.tensor_tensor(h3pre[:], h3ps[0:64, 0:324], h3tmpB[:],
                                        ALU.add)
                nc.scalar.activation(h3sb[:], h3pre[:], AF.Tanh,
                                     scale=1.0 / S3)

            # ---------------- L4 (redundant) ----------------
            with tc.tile_pool(name="ptp3", bufs=2, space="PSUM") as ptp3, \
                 tc.tile_pool(name="pl4", bufs=1, space="PSUM") as pl4:
                for t in range(3):
                    w = 128 if t < 2 else 324 - 256  # 68
                    tp = ptp3.tile([128, 64], dt.bfloat16, tag="tp3")
                    nc.tensor.transpose(tp[0:w, :], h3sb[:, 128 * t:128 * t + w],
                                        ident[0:64, 0:64])
                    nc.vector.tensor_copy(h3T[0:w, t, :], tp[0:w, :])
                outps = pl4.tile([64, 512], dt.float32, tag="outps")
                for t in range(3):
                    nc.tensor.matmul(outps[:, 0:8], h3T[:, t, :], w4sb[:, t, :],
                                     start=(t == 0), stop=False)
                nc.tensor.matmul(outps[:, 0:8], ones128[0:1, 0:64], b4row[:],
                                 start=False, stop=True)
                nc.vector.tensor_copy(outsb[:], outps[:, 0:8])
                nc.scalar.dma_start(out_d[:, :], outsb[:])
            w1pool.__exit__(None, None, None)

    nc.compile()
    return nc


def _prep_inputs(x1, x2, w1, b1, w2, b2, w3, b3, w4, b4):
    x1f = np.asarray(x1, np.float32).reshape(N, C, HW).transpose(1, 0, 2)
    x2f = np.asarray(x2, np.float32).reshape(N, C, HW).transpose(1, 0, 2)
    x1q = np.ascontiguousarray(x1f * SX).astype(_e3)
    x2q = np.ascontiguousarray(x2f * SX).astype(_e3)
    w1 = np.asarray(w1, np.float32)
    w2 = np.asarray(w2, np.float32)
    w3 = np.asarray(w3, np.float32)
    w4 = np.asarray(w4, np.float32)
    b1 = np.asarray(b1, np.float32)
    b2 = np.asarray(b2, np.float32)
    b3 = np.asarray(b3, np.float32)
    b4 = np.asarray(b4, np.float32)

    w3pad = np.zeros((1408, 324), np.float32)
    w3pad[:1296] = w3 * S3
    w3t = np.ascontiguousarray(
        w3pad.reshape(11, 128, 324).transpose(1, 0, 2)).astype(_e3)
    w4pad = np.zeros((384, 8), np.float32)
    w4pad[:324] = w4
    w4t = np.ascontiguousarray(
        w4pad.reshape(3, 128, 8).transpose(1, 0, 2)).astype(_bf)
    b3r = (b3 * S3).astype(_bf).reshape(1, 324)
    b4r = b4.astype(_bf).reshape(1, 8)

    in_maps = []
    for core in range(NCORES):
        w1c = (w1[:, COLS1 * core:COLS1 * (core + 1)] * S1).reshape(HW, HW, COLS1)
        main = w1c[:, 0:128, :]
        res = w1c[:, 128:HW, :].reshape(18, 8, 16, COLS1).reshape(18, 128, COLS1)
        w1m = np.concatenate([main, res], axis=0)          # [162, 128, 648]
        w1t = np.ascontiguousarray(w1m.transpose(1, 0, 2)).astype(_e3)

        w2pad = np.zeros((768, 1296), np.float32)
        w2pad[:COLS1] = w2[COLS1 * core:COLS1 * (core + 1)] * S2
        w2t = np.ascontiguousarray(
            w2pad.reshape(6, 128, 1296).transpose(1, 0, 2)).astype(_e3)

        in_maps.append({
            "x1q": x1q, "x2q": x2q,
            "w1t": w1t,
            "b1r": (b1[COLS1 * core:COLS1 * (core + 1)] * S1).astype(_bf).reshape(1, COLS1),
            "w2t": w2t,
            "b2r": (b2 * S2 / NCORES).astype(_bf).reshape(1, 1296),
            "w3t": w3t, "b3r": b3r,
            "w4t": w4t, "b4r": b4r,
        })
    return in_maps


def kernel(x1, x2, w1, b1, w2, b2, w3, b3, w4, b4):
    global LAST_RESULT
    from concourse.bass_utils import run_bass_kernel_spmd

    if "nc" not in _CACHE:
        _CACHE["nc"] = _build_nc()
    nc = _CACHE["nc"]

    in_maps = _prep_inputs(x1, x2, w1, b1, w2, b2, w3, b3, w4, b4)
    trace = bool(int(os.environ.get("HNET_TRACE", "0")))
    res = run_bass_kernel_spmd(nc, in_maps, core_ids=list(range(NCORES)),
                               trace=trace)
    LAST_RESULT = res
    H = np.asarray(res.results[0]["out"], np.float32)
    ones = np.ones((N, 1), np.float32)
    return np.concatenate([H, ones], axis=1).reshape(N, 3, 3)
